# revision 12
# baseline (speedup 1.0000x reference)
"""Bahdanau attention kernel for 8 trn2 NeuronCores.

Shards batch B=32 across 8 cores (4 batches/core); W1/W2/V replicated.
Per core, a single pass over img_tensor:
  - img tile [128t x 1024f] is PE-transposed to [f, t] chunks
  - score^T[u, t] = W1^T @ img^T via fp32r matmuls (full-rate, N=512)
  - tanh with per-partition bias (b1 + b2 + hidden@W2, pre-transposed to [u,b])
  - logits[1, t] = V^T @ tanh(score) + mask penalty (extra K=1 matmul)
  - exp without max-subtraction (logits bounded by ||V||_1 ~ 8)
  - context accumulated in PSUM: w[t,1] stationary vs resident natural img tile
"""

import numpy as np

import concourse.bass as bass
import concourse.mybir as mybir
from concourse import tile
from concourse.masks import make_identity
from concourse.vector_clock import ScopedClock

FP32 = mybir.dt.float32
FP32R = mybir.dt.float32r
AF = mybir.ActivationFunctionType

B, T, FEAT, UNITS, HID = 32, 2048, 1024, 512, 512
N_CORES = 8
B_LOC = B // N_CORES          # 4 batches per core
TT = 512                      # T tile size
N_TT = T // TT                # 4 T-tiles per batch
TC = TT // 128                # 4 128-row chunks per T tile
FC = FEAT // 128              # 8 feature chunks
UC = UNITS // 128             # 4 unit chunks


class CompatTileContext(tile.TileContext):
    """The walrus build here can't encode sem waits on a Drain (CoreV3
    CTRL_NO_STRUCT); emit the final-drain waits on individual NOPs."""

    def _drain_and_barrier(self, tick_clock, wait_clock):
        probe = self.nc.sync.nop(nofuse=True, hint="final_drain_waits")
        wait_clock.add_sem_waits(
            probe.ins, ScopedClock({None: tick_clock.global_clock})
        )
        waits = list(probe.ins.sync_info.on_wait or [])
        probe.ins.sync_info.on_wait = waits[:1]
        for w in waits[1:]:
            n = self.nc.sync.nop(nofuse=True, hint="final_drain_waits")
            if n.ins.sync_info is None:
                n.ins.sync_info = mybir.SyncInfo(on_wait=[w], on_update=[])
            else:
                n.ins.sync_info.on_wait = [w]
        self.nc.sync.drain()
        self.nc.all_engine_barrier()
        assert self.sems is not None
        popped = self.nc._tile_sem_poison_stack.pop()
        assert popped is self._sem_poison
        self.nc.clear_and_free_semaphores(list(self.sems.allocated().values()))
        self.nc.all_engine_barrier()


def build_kernel():
    nc = bass.Bass("TRN2", target_bir_lowering=False, debug=False,
                   num_devices=N_CORES)

    img = nc.dram_tensor("img_tensor", [B_LOC, T, FEAT], FP32R,
                         kind="ExternalInput").ap()
    mask = nc.dram_tensor("mask", [B_LOC, T], mybir.dt.int32,
                          kind="ExternalInput").ap()
    hidden = nc.dram_tensor("hidden", [B_LOC, HID], FP32,
                            kind="ExternalInput").ap()
    W1 = nc.dram_tensor("W1", [FEAT, UNITS], FP32R, kind="ExternalInput").ap()
    b1 = nc.dram_tensor("b1", [UNITS], FP32, kind="ExternalInput").ap()
    W2 = nc.dram_tensor("W2", [HID, UNITS], FP32R, kind="ExternalInput").ap()
    b2 = nc.dram_tensor("b2", [UNITS], FP32, kind="ExternalInput").ap()
    V = nc.dram_tensor("V", [UNITS, 1], FP32R, kind="ExternalInput").ap()
    bV = nc.dram_tensor("bV", [1], FP32, kind="ExternalInput").ap()
    ctx_out = nc.dram_tensor("context_vector", [B_LOC, FEAT], FP32,
                             kind="ExternalOutput").ap()
    attn_out = nc.dram_tensor("attention_weights", [B_LOC, T, 1], FP32,
                              kind="ExternalOutput").ap()

    with CompatTileContext(nc) as tc:
        consts = tc.alloc_tile_pool(name="consts", bufs=1)
        psum_tr = tc.alloc_tile_pool(name="psum_tr", bufs=2, space="PSUM")
        psum_s = tc.alloc_tile_pool(name="psum_s", bufs=2, space="PSUM")
        psum_small = tc.alloc_tile_pool(name="psum_small", bufs=2, space="PSUM")
        psum_ctx_pool = tc.alloc_tile_pool(name="psum_ctx", bufs=1, space="PSUM")
        imgp = tc.alloc_tile_pool(name="imgp", bufs=3)
        imgtp = tc.alloc_tile_pool(name="imgtp", bufs=2)
        tanhp = tc.alloc_tile_pool(name="tanhp", bufs=2)
        smalls = tc.alloc_tile_pool(name="smalls", bufs=2)
        perb = tc.alloc_tile_pool(name="perb", bufs=2)

        # ---------------- constants / preamble ----------------
        identity = consts.tile([128, 128], FP32)
        make_identity(nc, identity)
        identity_r = consts.tile([128, 128], FP32R)
        nc.vector.tensor_copy(out=identity_r, in_=identity)

        w1_sb = consts.tile([128, FC, UNITS], FP32R)
        nc.sync.dma_start(out=w1_sb, in_=W1.rearrange("(c p) u -> p c u", p=128))
        w2_sb = consts.tile([128, HID // 128, UNITS], FP32R)
        nc.sync.dma_start(out=w2_sb, in_=W2.rearrange("(c p) u -> p c u", p=128))
        v_sb = consts.tile([128, UC], FP32R)
        nc.sync.dma_start(out=v_sb, in_=V.rearrange("(c p) o -> p (c o)", p=128))

        ones_col_f = consts.tile([128, 1], FP32)
        nc.vector.memset(ones_col_f, 1.0)
        ones_col = consts.tile([128, 1], FP32R)
        nc.vector.tensor_copy(out=ones_col, in_=ones_col_f)
        ones_row_f = consts.tile([1, 128], FP32)
        nc.vector.memset(ones_row_f, 1.0)
        ones_row = consts.tile([1, 128], FP32R)
        nc.vector.tensor_copy(out=ones_row, in_=ones_row_f)

        b1_sb = consts.tile([1, UNITS], FP32)
        nc.sync.dma_start(out=b1_sb, in_=b1.unsqueeze(0))
        b2_sb = consts.tile([1, UNITS], FP32)
        nc.sync.dma_start(out=b2_sb, in_=b2.unsqueeze(0))
        b12 = consts.tile([1, UNITS], FP32R)
        nc.vector.tensor_add(out=b12, in0=b1_sb, in1=b2_sb)

        bv4 = consts.tile([B_LOC, 1], FP32)
        nc.sync.dma_start(out=bv4, in_=bV.unsqueeze(0).to_broadcast([B_LOC, 1]))

        # mask -> additive penalty  pen = (mask-1)*1e9 + bV   [B_LOC, T]
        mask_sb = consts.tile([B_LOC, T], mybir.dt.int32)
        nc.sync.dma_start(out=mask_sb, in_=mask)
        maskf = consts.tile([B_LOC, T], FP32)
        nc.vector.tensor_copy(out=maskf, in_=mask_sb)
        pen = consts.tile([B_LOC, T], FP32)
        nc.scalar.activation(out=pen, in_=maskf, func=AF.Copy, scale=1e9, bias=-1e9)
        nc.vector.tensor_scalar_add(out=pen, in0=pen, scalar1=bv4)
        # matmul rhs needs base partition 0: move each row to partition 0
        pen_rows = consts.tile([1, B_LOC, T], FP32)
        for b in range(B_LOC):
            nc.sync.dma_start(out=pen_rows[:, b, :], in_=pen[b:b + 1, :])

        # hidden -> hidT [128h, hc, B_LOC]
        hid_sb = consts.tile([B_LOC, HID], FP32)
        nc.sync.dma_start(out=hid_sb, in_=hidden)
        psum_h = psum_small.tile([128, (HID // 128) * B_LOC], FP32, tag="small")
        for c in range(HID // 128):
            nc.tensor.transpose(
                psum_h[:, c * B_LOC:(c + 1) * B_LOC],
                hid_sb[:, c * 128:(c + 1) * 128],
                identity[0:B_LOC, 0:B_LOC],
            )
        hidT = consts.tile([128, HID // 128, B_LOC], FP32R)
        nc.vector.tensor_copy(out=hidT.rearrange("p c b -> p (c b)"), in_=psum_h)

        # cb[b, u] = b1 + b2 + hidden @ W2   (PE, bias via ones-row matmul)
        psum_cb = psum_small.tile([B_LOC, UNITS], FP32, tag="small")
        for c in range(HID // 128):
            nc.tensor.matmul(
                psum_cb,
                lhsT=hidT[:, c, :],
                rhs=w2_sb[:, c, :],
                start=(c == 0), stop=False,
            )
        nc.tensor.matmul(
            psum_cb,
            lhsT=ones_row[:, 0:B_LOC],
            rhs=b12,
            start=False, stop=True,
        )
        cb_sb = consts.tile([B_LOC, UNITS], FP32)
        nc.vector.tensor_copy(out=cb_sb, in_=psum_cb)
        # transpose -> cbT [128u, uc, b]
        psum_cbt = psum_small.tile([128, UC * B_LOC], FP32, tag="small")
        for c in range(UC):
            nc.tensor.transpose(
                psum_cbt[:, c * B_LOC:(c + 1) * B_LOC],
                cb_sb[:, c * 128:(c + 1) * 128],
                identity[0:B_LOC, 0:B_LOC],
            )
        cbT = consts.tile([128, UC, B_LOC], FP32)
        nc.vector.tensor_copy(out=cbT.rearrange("p c b -> p (c b)"), in_=psum_cbt)

        # ---------------- main loop ----------------
        for b in range(B_LOC):
            w_sb = perb.tile([128, N_TT * TC], FP32R, tag="w_sb")
            psum_ctx = psum_ctx_pool.tile([1, FEAT], FP32)

            for tt in range(N_TT):
                t0 = tt * TT
                # load natural img tile [128, tc, f]
                img_sb = imgp.tile([128, TC, FEAT], FP32R, tag="img_sb")
                for c in range(TC):
                    nc.sync.dma_start(
                        out=img_sb[:, c, :],
                        in_=img[b, t0 + c * 128: t0 + (c + 1) * 128, :],
                    )
                # PE transpose -> imgT [128f, fc, t]
                imgT = imgtp.tile([128, FC, TT], FP32R, tag="imgT")
                for f in range(FC):
                    ptr = psum_tr.tile([128, TT], FP32R, tag="ptr")
                    for c in range(TC):
                        nc.tensor.transpose(
                            ptr[:, c * 128:(c + 1) * 128],
                            img_sb[:, c, f * 128:(f + 1) * 128],
                            identity_r,
                        )
                    nc.vector.tensor_copy(out=imgT[:, f, :], in_=ptr)

                # score^T [u, t] per u-chunk; tanh with per-partition bias
                tanh_sb = tanhp.tile([128, UC, TT], FP32R, tag="tanh_sb")
                for u in range(UC):
                    ps = psum_s.tile([128, TT], FP32, tag="ps")
                    for f in range(FC):
                        nc.tensor.matmul(
                            ps,
                            lhsT=w1_sb[:, f, u * 128:(u + 1) * 128],
                            rhs=imgT[:, f, :],
                            start=(f == 0), stop=(f == FC - 1),
                        )
                    nc.scalar.activation(
                        out=tanh_sb[:, u, :], in_=ps, func=AF.Tanh,
                        bias=cbT[:, u, b:b + 1], scale=1.0,
                    )

                # logits [1, TT] = V^T @ tanh + pen
                pl = psum_small.tile([1, TT], FP32, tag="small")
                for u in range(UC):
                    nc.tensor.matmul(
                        pl,
                        lhsT=v_sb[:, u:u + 1],
                        rhs=tanh_sb[:, u, :],
                        start=(u == 0), stop=(u == UC - 1),
                    )
                logit_row = smalls.tile([1, TT], FP32, tag="logit_row")
                nc.vector.tensor_add(out=logit_row, in0=pl,
                                     in1=pen_rows[:, b, t0:t0 + TT])

                # transpose logits to [128, TC] and exp into w_sb columns
                pw = psum_small.tile([128, TC], FP32, tag="small")
                for c in range(TC):
                    nc.tensor.transpose(
                        pw[:, c:c + 1],
                        logit_row[:, c * 128:(c + 1) * 128],
                        identity[0:1, 0:1],
                    )
                nc.scalar.activation(
                    out=w_sb[:, tt * TC:(tt + 1) * TC], in_=pw, func=AF.Exp,
                )

                # context accumulation: w chunk stationary vs natural img tile
                for c in range(TC):
                    first = (tt == 0 and c == 0)
                    last = (tt == N_TT - 1 and c == TC - 1)
                    for h in range(2):
                        nc.tensor.matmul(
                            psum_ctx[:, h * 512:(h + 1) * 512],
                            lhsT=w_sb[:, tt * TC + c: tt * TC + c + 1],
                            rhs=img_sb[:, c, h * 512:(h + 1) * 512],
                            start=first, stop=last,
                        )

            # ---- batch epilogue: normalize ----
            psr = psum_small.tile([1, N_TT * TC], FP32, tag="small")
            nc.tensor.matmul(
                psr, lhsT=ones_col, rhs=w_sb,
                start=True, stop=True,
            )
            s_sc = smalls.tile([1, 1], FP32, tag="s_sc")
            nc.vector.reduce_sum(out=s_sc, in_=psr, axis=mybir.AxisListType.X)
            r_sc = smalls.tile([1, 1], FP32, tag="r_sc")
            nc.vector.reciprocal(out=r_sc, in_=s_sc)

            ctx_sb = perb.tile([1, FEAT], FP32, tag="ctx_sb")
            nc.vector.tensor_scalar_mul(out=ctx_sb, in0=psum_ctx, scalar1=r_sc)
            nc.sync.dma_start(out=ctx_out[b, :].unsqueeze(0), in_=ctx_sb)

            # broadcast 1/S to 128 partitions via PE
            pbc = psum_small.tile([128, 1], FP32, tag="small")
            nc.tensor.matmul(
                pbc, lhsT=ones_row_f, rhs=r_sc,
                start=True, stop=True,
            )
            wn = perb.tile([128, N_TT * TC], FP32, tag="wn")
            nc.vector.tensor_scalar_mul(out=wn, in0=w_sb, scalar1=pbc)
            pwt = psum_small.tile([N_TT * TC, 128], FP32, tag="small")
            nc.tensor.transpose(pwt, wn, identity)
            wnT = perb.tile([N_TT * TC, 128], FP32, tag="wnT")
            nc.vector.tensor_copy(out=wnT, in_=pwt)
            nc.sync.dma_start(
                out=attn_out[b].rearrange("(c p) o -> c (p o)", p=128), in_=wnT,
            )

        for p in (perb, smalls, tanhp, imgtp, imgp, psum_ctx_pool,
                  psum_small, psum_s, psum_tr, consts):
            p.release()

    _split_sync_waits(nc)
    return nc


def _split_sync_waits(nc):
    """The walrus build in this container encodes at most one sync wait per
    instruction (and none on Drain).  Move excess waits onto same-engine NOPs
    inserted immediately before the instruction."""
    f = nc.m.functions[0]
    uid = 0
    for bb in f.blocks:
        new_insts = []
        for inst in bb.instructions:
            si = getattr(inst, "sync_info", None)
            waits = list(si.on_wait) if si is not None and si.on_wait else []
            budget = 0 if "Drain" in type(inst).__name__ else 1
            if len(waits) > budget:
                keep = waits[len(waits) - budget:] if budget else []
                spill = waits[: len(waits) - budget]
                for w in spill:
                    uid += 1
                    nop = mybir.InstNoOp(
                        name=f"{inst.name}-sw{uid}",
                        sync_info=mybir.SyncInfo(on_wait=[w], on_update=[]),
                        bass_nofuse=True,
                        engine=inst.engine,
                    )
                    new_insts.append(nop)
                si.on_wait = keep
            new_insts.append(inst)
        if len(new_insts) != len(bb.instructions):
            bb.instructions = new_insts


_NC_CACHE = None


def _get_nc():
    global _NC_CACHE
    if _NC_CACHE is None:
        _NC_CACHE = build_kernel()
    return _NC_CACHE


def run_sharded(inputs, trace=False):
    """inputs: dict of full-size numpy arrays. Returns (results, extras)."""
    from concourse.bass_utils import run_bass_kernel_spmd

    nc = _get_nc()
    in_maps = []
    for i in range(N_CORES):
        s = slice(i * B_LOC, (i + 1) * B_LOC)
        in_maps.append({
            "img_tensor": np.ascontiguousarray(inputs["img_tensor"][s]),
            "mask": np.ascontiguousarray(inputs["mask"][s]),
            "hidden": np.ascontiguousarray(inputs["hidden"][s]),
            "W1": inputs["W1"], "b1": inputs["b1"],
            "W2": inputs["W2"], "b2": inputs["b2"],
            "V": inputs["V"], "bV": inputs["bV"],
        })
    res = run_bass_kernel_spmd(nc, in_maps, core_ids=list(range(N_CORES)),
                               trace=trace)
    ctx = np.concatenate([res.results[i]["context_vector"]
                          for i in range(N_CORES)], axis=0)
    attn = np.concatenate([res.results[i]["attention_weights"]
                           for i in range(N_CORES)], axis=0)
    return (ctx, attn), res


def kernel(**inputs):
    inputs = {k: np.asarray(v) for k, v in inputs.items()}
    (ctx, attn), _ = run_sharded(inputs, trace=False)
    return ctx, attn


# revision 13
# speedup vs baseline: 7.3931x; 7.3931x over previous
"""Bahdanau attention kernel for 8 trn2 NeuronCores.

Shards batch B=32 across 8 cores (4 batches/core); W1/W2/V replicated.
Per core, a single pass over img_tensor:
  - img tile [128t x 1024f] is PE-transposed to [f, t] chunks
  - score^T[u, t] = W1^T @ img^T via fp32r matmuls (full-rate, N=512)
  - tanh with per-partition bias (b1 + b2 + hidden@W2, pre-transposed to [u,b])
  - logits[1, t] = V^T @ tanh(score) + mask penalty (extra K=1 matmul)
  - exp without max-subtraction (logits bounded by ||V||_1 ~ 8)
  - context accumulated in PSUM: w[t,1] stationary vs resident natural img tile
"""

import numpy as np

import concourse.bass as bass
import concourse.mybir as mybir
from concourse import tile
from concourse.masks import make_identity
from concourse.vector_clock import ScopedClock

FP32 = mybir.dt.float32
FP32R = mybir.dt.float32r
AF = mybir.ActivationFunctionType

B, T, FEAT, UNITS, HID = 32, 2048, 1024, 512, 512
N_CORES = 8
B_LOC = B // N_CORES          # 4 batches per core
TT = 512                      # T tile size
N_TT = T // TT                # 4 T-tiles per batch
TC = TT // 128                # 4 128-row chunks per T tile
FC = FEAT // 128              # 8 feature chunks
UC = UNITS // 128             # 4 unit chunks


class CompatTileContext(tile.TileContext):
    """The walrus build here can't encode sem waits on a Drain (CoreV3
    CTRL_NO_STRUCT); emit the final-drain waits on individual NOPs."""

    def _drain_and_barrier(self, tick_clock, wait_clock):
        probe = self.nc.sync.nop(nofuse=True, hint="final_drain_waits")
        wait_clock.add_sem_waits(
            probe.ins, ScopedClock({None: tick_clock.global_clock})
        )
        waits = list(probe.ins.sync_info.on_wait or [])
        probe.ins.sync_info.on_wait = waits[:1]
        for w in waits[1:]:
            n = self.nc.sync.nop(nofuse=True, hint="final_drain_waits")
            if n.ins.sync_info is None:
                n.ins.sync_info = mybir.SyncInfo(on_wait=[w], on_update=[])
            else:
                n.ins.sync_info.on_wait = [w]
        self.nc.sync.drain()
        self.nc.all_engine_barrier()
        assert self.sems is not None
        popped = self.nc._tile_sem_poison_stack.pop()
        assert popped is self._sem_poison
        self.nc.clear_and_free_semaphores(list(self.sems.allocated().values()))
        self.nc.all_engine_barrier()


def build_kernel(split_waits=True):
    nc = bass.Bass("TRN2", target_bir_lowering=False, debug=False,
                   num_devices=N_CORES)

    img = nc.dram_tensor("img_tensor", [B_LOC, T, FEAT], FP32R,
                         kind="ExternalInput").ap()
    mask = nc.dram_tensor("mask", [B_LOC, T], mybir.dt.int32,
                          kind="ExternalInput").ap()
    hidden = nc.dram_tensor("hidden", [B_LOC, HID], FP32,
                            kind="ExternalInput").ap()
    W1 = nc.dram_tensor("W1", [FEAT, UNITS], FP32R, kind="ExternalInput").ap()
    b1 = nc.dram_tensor("b1", [UNITS], FP32, kind="ExternalInput").ap()
    W2 = nc.dram_tensor("W2", [HID, UNITS], FP32R, kind="ExternalInput").ap()
    b2 = nc.dram_tensor("b2", [UNITS], FP32, kind="ExternalInput").ap()
    V = nc.dram_tensor("V", [UNITS, 1], FP32R, kind="ExternalInput").ap()
    bV = nc.dram_tensor("bV", [1], FP32, kind="ExternalInput").ap()
    ctx_out = nc.dram_tensor("context_vector", [B_LOC, FEAT], FP32,
                             kind="ExternalOutput").ap()
    attn_out = nc.dram_tensor("attention_weights", [B_LOC, T, 1], FP32,
                              kind="ExternalOutput").ap()

    with CompatTileContext(nc) as tc:
        consts = tc.alloc_tile_pool(name="consts", bufs=1)
        psum_tr = tc.alloc_tile_pool(name="psum_tr", bufs=2, space="PSUM")
        psum_s = tc.alloc_tile_pool(name="psum_s", bufs=2, space="PSUM")
        psum_small = tc.alloc_tile_pool(name="psum_small", bufs=2, space="PSUM")
        psum_ctx_pool = tc.alloc_tile_pool(name="psum_ctx", bufs=1, space="PSUM")
        imgp = tc.alloc_tile_pool(name="imgp", bufs=3)
        imgtp = tc.alloc_tile_pool(name="imgtp", bufs=2)
        tanhp = tc.alloc_tile_pool(name="tanhp", bufs=2)
        smalls = tc.alloc_tile_pool(name="smalls", bufs=2)
        perb = tc.alloc_tile_pool(name="perb", bufs=2)

        # ---------------- constants / preamble ----------------
        identity = consts.tile([128, 128], FP32)
        make_identity(nc, identity)
        identity_r = consts.tile([128, 128], FP32R)
        nc.vector.tensor_copy(out=identity_r, in_=identity)

        w1_sb = consts.tile([128, FC, UNITS], FP32R)
        nc.sync.dma_start(out=w1_sb, in_=W1.rearrange("(c p) u -> p c u", p=128))
        w2_sb = consts.tile([128, HID // 128, UNITS], FP32R)
        nc.sync.dma_start(out=w2_sb, in_=W2.rearrange("(c p) u -> p c u", p=128))
        v_sb = consts.tile([128, UC], FP32R)
        nc.sync.dma_start(out=v_sb, in_=V.rearrange("(c p) o -> p (c o)", p=128))

        ones_col_f = consts.tile([128, 1], FP32)
        nc.vector.memset(ones_col_f, 1.0)
        ones_col = consts.tile([128, 1], FP32R)
        nc.vector.tensor_copy(out=ones_col, in_=ones_col_f)
        ones_row_f = consts.tile([1, 128], FP32)
        nc.vector.memset(ones_row_f, 1.0)
        ones_row = consts.tile([1, 128], FP32R)
        nc.vector.tensor_copy(out=ones_row, in_=ones_row_f)

        b1_sb = consts.tile([1, UNITS], FP32)
        nc.sync.dma_start(out=b1_sb, in_=b1.unsqueeze(0))
        b2_sb = consts.tile([1, UNITS], FP32)
        nc.sync.dma_start(out=b2_sb, in_=b2.unsqueeze(0))
        b12 = consts.tile([1, UNITS], FP32R)
        nc.vector.tensor_add(out=b12, in0=b1_sb, in1=b2_sb)

        bv4 = consts.tile([B_LOC, 1], FP32)
        nc.sync.dma_start(out=bv4, in_=bV.unsqueeze(0).to_broadcast([B_LOC, 1]))

        # mask -> additive penalty  pen = (mask-1)*1e9 + bV   [B_LOC, T]
        mask_sb = consts.tile([B_LOC, T], mybir.dt.int32)
        nc.sync.dma_start(out=mask_sb, in_=mask)
        maskf = consts.tile([B_LOC, T], FP32)
        nc.vector.tensor_copy(out=maskf, in_=mask_sb)
        pen = consts.tile([B_LOC, T], FP32)
        nc.scalar.activation(out=pen, in_=maskf, func=AF.Copy, scale=1e9, bias=-1e9)
        nc.vector.tensor_scalar_add(out=pen, in0=pen, scalar1=bv4)
        # matmul rhs needs base partition 0: move each row to partition 0
        pen_rows = consts.tile([1, B_LOC, T], FP32)
        for b in range(B_LOC):
            nc.sync.dma_start(out=pen_rows[:, b, :], in_=pen[b:b + 1, :])

        # hidden -> hidT [128h, hc, B_LOC]
        hid_sb = consts.tile([B_LOC, HID], FP32)
        nc.sync.dma_start(out=hid_sb, in_=hidden)
        psum_h = psum_small.tile([128, (HID // 128) * B_LOC], FP32, tag="small")
        for c in range(HID // 128):
            nc.tensor.transpose(
                psum_h[:, c * B_LOC:(c + 1) * B_LOC],
                hid_sb[:, c * 128:(c + 1) * 128],
                identity[0:B_LOC, 0:B_LOC],
            )
        hidT = consts.tile([128, HID // 128, B_LOC], FP32R)
        nc.vector.tensor_copy(out=hidT.rearrange("p c b -> p (c b)"), in_=psum_h)

        # cb[b, u] = b1 + b2 + hidden @ W2   (PE, bias via ones-row matmul)
        psum_cb = psum_small.tile([B_LOC, UNITS], FP32, tag="small")
        for c in range(HID // 128):
            nc.tensor.matmul(
                psum_cb,
                lhsT=hidT[:, c, :],
                rhs=w2_sb[:, c, :],
                start=(c == 0), stop=False,
            )
        nc.tensor.matmul(
            psum_cb,
            lhsT=ones_row[:, 0:B_LOC],
            rhs=b12,
            start=False, stop=True,
        )
        cb_sb = consts.tile([B_LOC, UNITS], FP32)
        nc.vector.tensor_copy(out=cb_sb, in_=psum_cb)
        # transpose -> cbT [128u, uc, b]
        psum_cbt = psum_small.tile([128, UC * B_LOC], FP32, tag="small")
        for c in range(UC):
            nc.tensor.transpose(
                psum_cbt[:, c * B_LOC:(c + 1) * B_LOC],
                cb_sb[:, c * 128:(c + 1) * 128],
                identity[0:B_LOC, 0:B_LOC],
            )
        cbT = consts.tile([128, UC, B_LOC], FP32)
        nc.vector.tensor_copy(out=cbT.rearrange("p c b -> p (c b)"), in_=psum_cbt)

        # ---------------- main loop ----------------
        for b in range(B_LOC):
            w_sb = perb.tile([128, N_TT * TC], FP32R, tag="w_sb")
            psum_ctx = psum_ctx_pool.tile([1, FEAT], FP32)

            for tt in range(N_TT):
                t0 = tt * TT
                # load natural img tile [128, tc, f]
                img_sb = imgp.tile([128, TC, FEAT], FP32R, tag="img_sb")
                for c in range(TC):
                    nc.sync.dma_start(
                        out=img_sb[:, c, :],
                        in_=img[b, t0 + c * 128: t0 + (c + 1) * 128, :],
                    )
                # PE transpose -> imgT [128f, fc, t]
                imgT = imgtp.tile([128, FC, TT], FP32R, tag="imgT")
                for f in range(FC):
                    ptr = psum_tr.tile([128, TT], FP32R, tag="ptr")
                    for c in range(TC):
                        nc.tensor.transpose(
                            ptr[:, c * 128:(c + 1) * 128],
                            img_sb[:, c, f * 128:(f + 1) * 128],
                            identity_r,
                        )
                    nc.vector.tensor_copy(out=imgT[:, f, :], in_=ptr)

                # score^T [u, t] per u-chunk; tanh with per-partition bias
                tanh_sb = tanhp.tile([128, UC, TT], FP32R, tag="tanh_sb")
                for u in range(UC):
                    ps = psum_s.tile([128, TT], FP32, tag="ps")
                    for f in range(FC):
                        nc.tensor.matmul(
                            ps,
                            lhsT=w1_sb[:, f, u * 128:(u + 1) * 128],
                            rhs=imgT[:, f, :],
                            start=(f == 0), stop=(f == FC - 1),
                        )
                    nc.scalar.activation(
                        out=tanh_sb[:, u, :], in_=ps, func=AF.Tanh,
                        bias=cbT[:, u, b:b + 1], scale=1.0,
                    )

                # logits [1, TT] = V^T @ tanh + pen
                pl = psum_small.tile([1, TT], FP32, tag="small")
                for u in range(UC):
                    nc.tensor.matmul(
                        pl,
                        lhsT=v_sb[:, u:u + 1],
                        rhs=tanh_sb[:, u, :],
                        start=(u == 0), stop=(u == UC - 1),
                    )
                logit_row = smalls.tile([1, TT], FP32, tag="logit_row")
                nc.vector.tensor_add(out=logit_row, in0=pl,
                                     in1=pen_rows[:, b, t0:t0 + TT])

                # transpose logits to [128, TC] and exp into w_sb columns
                pw = psum_small.tile([128, TC], FP32, tag="small")
                for c in range(TC):
                    nc.tensor.transpose(
                        pw[:, c:c + 1],
                        logit_row[:, c * 128:(c + 1) * 128],
                        identity[0:1, 0:1],
                    )
                nc.scalar.activation(
                    out=w_sb[:, tt * TC:(tt + 1) * TC], in_=pw, func=AF.Exp,
                )

                # context accumulation: w chunk stationary vs natural img tile
                for c in range(TC):
                    first = (tt == 0 and c == 0)
                    last = (tt == N_TT - 1 and c == TC - 1)
                    for h in range(2):
                        nc.tensor.matmul(
                            psum_ctx[:, h * 512:(h + 1) * 512],
                            lhsT=w_sb[:, tt * TC + c: tt * TC + c + 1],
                            rhs=img_sb[:, c, h * 512:(h + 1) * 512],
                            start=first, stop=last,
                        )

            # ---- batch epilogue: normalize ----
            psr = psum_small.tile([1, N_TT * TC], FP32, tag="small")
            nc.tensor.matmul(
                psr, lhsT=ones_col, rhs=w_sb,
                start=True, stop=True,
            )
            s_sc = smalls.tile([1, 1], FP32, tag="s_sc")
            nc.vector.reduce_sum(out=s_sc, in_=psr, axis=mybir.AxisListType.X)
            r_sc = smalls.tile([1, 1], FP32, tag="r_sc")
            nc.vector.reciprocal(out=r_sc, in_=s_sc)

            ctx_sb = perb.tile([1, FEAT], FP32, tag="ctx_sb")
            nc.vector.tensor_scalar_mul(out=ctx_sb, in0=psum_ctx, scalar1=r_sc)
            nc.sync.dma_start(out=ctx_out[b, :].unsqueeze(0), in_=ctx_sb)

            # broadcast 1/S to 128 partitions via PE
            pbc = psum_small.tile([128, 1], FP32, tag="small")
            nc.tensor.matmul(
                pbc, lhsT=ones_row_f, rhs=r_sc,
                start=True, stop=True,
            )
            wn = perb.tile([128, N_TT * TC], FP32, tag="wn")
            nc.vector.tensor_scalar_mul(out=wn, in0=w_sb, scalar1=pbc)
            pwt = psum_small.tile([N_TT * TC, 128], FP32, tag="small")
            nc.tensor.transpose(pwt, wn, identity)
            wnT = perb.tile([N_TT * TC, 128], FP32, tag="wnT")
            nc.vector.tensor_copy(out=wnT, in_=pwt)
            nc.sync.dma_start(
                out=attn_out[b].rearrange("(c p) o -> c (p o)", p=128), in_=wnT,
            )

        for p in (perb, smalls, tanhp, imgtp, imgp, psum_ctx_pool,
                  psum_small, psum_s, psum_tr, consts):
            p.release()

    if split_waits:
        _split_sync_waits(nc)
    return nc


def _split_sync_waits(nc):
    """The walrus build in this container encodes at most one sync wait per
    instruction (and none on Drain).  Move excess waits onto same-engine NOPs
    inserted immediately before the instruction."""
    f = nc.m.functions[0]
    uid = 0
    for bb in f.blocks:
        new_insts = []
        for inst in bb.instructions:
            si = getattr(inst, "sync_info", None)
            waits = list(si.on_wait) if si is not None and si.on_wait else []
            budget = 0 if "Drain" in type(inst).__name__ else 1
            if len(waits) > budget:
                keep = waits[len(waits) - budget:] if budget else []
                spill = waits[: len(waits) - budget]
                for w in spill:
                    uid += 1
                    nop = mybir.InstNoOp(
                        name=f"{inst.name}-sw{uid}",
                        sync_info=mybir.SyncInfo(on_wait=[w], on_update=[]),
                        bass_nofuse=True,
                        engine=inst.engine,
                    )
                    new_insts.append(nop)
                si.on_wait = keep
            new_insts.append(inst)
        if len(new_insts) != len(bb.instructions):
            bb.instructions = new_insts


_NC_CACHE = None


def _get_nc():
    global _NC_CACHE
    if _NC_CACHE is None:
        _NC_CACHE = build_kernel()
    return _NC_CACHE


def run_sharded(inputs, trace=False):
    """inputs: dict of full-size numpy arrays. Returns (results, extras)."""
    from concourse.bass_utils import run_bass_kernel_spmd

    nc = _get_nc()
    in_maps = []
    for i in range(N_CORES):
        s = slice(i * B_LOC, (i + 1) * B_LOC)
        in_maps.append({
            "img_tensor": np.ascontiguousarray(inputs["img_tensor"][s]),
            "mask": np.ascontiguousarray(inputs["mask"][s]),
            "hidden": np.ascontiguousarray(inputs["hidden"][s]),
            "W1": inputs["W1"], "b1": inputs["b1"],
            "W2": inputs["W2"], "b2": inputs["b2"],
            "V": inputs["V"], "bV": inputs["bV"],
        })
    res = run_bass_kernel_spmd(nc, in_maps, core_ids=list(range(N_CORES)),
                               trace=trace)
    ctx = np.concatenate([res.results[i]["context_vector"]
                          for i in range(N_CORES)], axis=0)
    attn = np.concatenate([res.results[i]["attention_weights"]
                           for i in range(N_CORES)], axis=0)
    return (ctx, attn), res


def kernel(**inputs):
    inputs = {k: np.asarray(v) for k, v in inputs.items()}
    (ctx, attn), _ = run_sharded(inputs, trace=False)
    return ctx, attn
